# revision 35
# baseline (speedup 1.0000x reference)
"""Trainium2 Bass kernel for RelPatchAttention2D (THW) — fp8 DoubleRow version.

Problem: q,k,v (4,16,16,128,128) f32. Patchify into 4096 patches/batch of
dim 1024. sim[q,k] = (qk+s)/(qq+kk-qk+s); tqk[k] = mean_q sim; out = tqk * v.

Sharding (no collectives): 8 cores = 4 batches x 2 key-halves. Each core:
full queries (4096) x its 2048 keys.

Per-core kernel, keys on partitions / queries on free dim, kt (128-key
block) outer, qt (512-query block) inner:
  PE:    a PURE stream of fp8-e4m3 DoubleRow matmuls (4 per tile,
         256-contraction each) accumulating P = -qk into PSUM.
  GPSIMD: A_kt = qhat_bcast + khat[kt]  (tensor_scalar add, f32) — the
         denominator offset, built on the otherwise-idle engine.
  DVE:   ONE fused custom op per tile:
           d  = P + A            (= qq+kk-qk+s = D, f32)
           r  = recip_1NR(d)     (bitwise-NOT seed + 1 Newton pass,
                                  constants passed per-partition at runtime)
           accum += P * r        (= -qk*r; sign folded into final scale)
  tqk = rowsum(acc) * (-1/4096) + corr;  out = v * tqk  (ACT scale).

Numerics: host quantizes q,k to fp8-e4m3 and corrects tqk to first+second
order in the quantization residuals (c1+c2+c3), plus a sampled per-key
correction for the approximate reciprocal (c5). Validated ~3e-3 rel err
vs f64 reference (gate 2e-2).
"""
import sys

import numpy as np

sys.path.insert(0, '/opt/trn_rl_repo')

SMOOTH = 1e-05
B, T, C, H, W = 4, 16, 16, 128, 128
SH = SW = 16
PH = PW = 8
NPATCH = T * SH * SW          # 4096 queries per batch
DPATCH = C * PH * PW          # 1024
KEYS = NPATCH // 2            # 2048 keys per core
N_CORES = 8

QT = NPATCH // 512            # 8 query tiles of 512
KT = KEYS // 128              # 16 key tiles of 128
DC = DPATCH // 128            # 8 contraction chunks of 128
DCP = DC // 2                 # 4 DoubleRow pairs
NSAMP = 768                   # rows sampled for the recip correction

_OP_NAME = "SIM_DNR_MAC_ANT"


# ------------------------------------------------------- custom DVE op

def _register_fused_op():
    """Register accum += Src0 * recip_1NR(Src0 + Src1) as a custom DVE op.

    In-process extension of the dve_ops registry (same mechanism as adding
    the op to dve_ops.py; nothing on disk is modified).
    C0 = Newton constant (~2.0), C1 = seed scale, both per-partition APs.
    """
    from operator import add as _add

    import concourse.dve_ops as dops
    from concourse.dve_spec import (
        AluOp, Bin, Spec, Src0, Src1, Zero, C0, C1, lower, _has_src1,
    )
    from concourse.dve_uop import DveOpSpec

    for o in dops.OPS:
        if o.name == _OP_NAME:
            return o

    _d = Src0 + Src1
    _not = Bin(AluOp.BITWISE_NOT, _d, _d)
    _y0 = _not * C1
    _y1 = _y0 * (C0 - _d * _y0)

    def _ref(in0, in1, c0, c1, c2):
        p = np.asarray(in0, np.float32)
        d = (p + np.asarray(in1, np.float32)).astype(np.float32)
        nx = (~d.view(np.int32)).view(np.float32)
        y0 = (nx * np.asarray(c1, np.float32)).astype(np.float32)
        y1 = (y0 * (np.asarray(c0, np.float32) - d * y0).astype(np.float32)
              ).astype(np.float32)
        b = (p * y1).astype(np.float32)
        return b, b.reshape(b.shape[0], -1).sum(-1, keepdims=True).astype(np.float32)

    spec = Spec(body=Src0 * _y1, accum=_add, accum_init=Zero, reference=_ref)
    row = dops._CUSTOM_DVE_ROW_BASE + len(dops.OPS)
    shas = {}
    for ver in ("v3", "v4"):
        s = DveOpSpec(name=_OP_NAME, opcode=row,
                      uops=lower(spec, ver=ver), rd1_en=_has_src1(spec))
        shas[ver] = s.sha(ver)
    op = dops.DveOp(_OP_NAME, spec, subdim=False, uops_sha=shas)
    dops.OPS.append(op)
    dops.CUSTOM_DVE_SPECS[_OP_NAME] = spec
    dops._SUB_OPCODE_FOR_NAME[_OP_NAME] = row
    return op


# ----------------------------------------------------------------- host side

def _patchify_mat(x):
    # (B,T,C,H,W) -> (B, 4096, 1024), patch index = ((t*16+sh)*16+sw)
    xp = x.reshape(B, T, C, SH, PH, SW, PW).transpose(0, 1, 3, 5, 2, 4, 6)
    return np.ascontiguousarray(xp).reshape(B, NPATCH, DPATCH)


def _unpatchify_mat(p):
    x = p.reshape(B, T, SH, SW, C, PH, PW).transpose(0, 1, 4, 2, 5, 3, 6)
    return np.ascontiguousarray(x).reshape(B, T, C, H, W)


def _recip_1nr(x32, c0, c1):
    x = np.asarray(x32, np.float32)
    nx = (~x.view(np.int32)).view(np.float32)
    y0 = (nx * np.float32(c0)).astype(np.float32)
    return (y0 * (np.float32(c1) - x * y0).astype(np.float32)).astype(np.float32)


def _optimize_recip_consts(d_samples):
    """(c0,c1) minimizing max |x*y1-1|. x*y1 = u*(c1-u), u = c0*x*bitcast(~x);
    concave in u so only the z-range endpoints + vertex matter."""
    x = np.asarray(d_samples, np.float32)
    nx = (~x.view(np.int32)).view(np.float32)
    z = x.astype(np.float64) * nx.astype(np.float64)
    zmin, zmax = z.min(), z.max()

    def err(c0, c1):
        us = [c0 * zmin, c0 * zmax]
        lo, hi = min(us), max(us)
        cand = [lo, hi] + ([c1 / 2] if lo < c1 / 2 < hi else [])
        return max(abs(u * (c1 - u) - 1) for u in cand)

    best = None
    for c0 in np.linspace(-1 / abs(zmin), -1 / abs(zmax), 400):
        for c1 in np.linspace(1.95, 2.1, 300):
            e = err(c0, c1)
            if best is None or e < best[0]:
                best = (e, c0, c1)
    _, bc0, bc1 = best
    for c0 in np.linspace(bc0 * 1.01, bc0 * 0.99, 160):
        for c1 in np.linspace(bc1 - 0.004, bc1 + 0.004, 160):
            e = err(c0, c1)
            if e < best[0]:
                best = (e, c0, c1)
    return best[1], best[2]


def _host_prepare(q, k, v):
    import ml_dtypes
    F8 = ml_dtypes.float8_e4m3

    QP = _patchify_mat(q)
    KP = _patchify_mat(k)
    VP = _patchify_mat(v)

    rng = np.random.default_rng(12345)
    in_maps = []
    consts = None
    for b in range(B):
        q8f = QP[b].astype(F8)
        q8 = q8f.astype(np.float32)
        qq = np.square(q8, dtype=np.float64).sum(-1)
        qhat = (qq + SMOOTH).astype(np.float32)       # f32, exact on device
        # moving tensor: qta[p, c*4096+i] = q8[i, c*128+p]
        qta = np.ascontiguousarray(
            q8f.reshape(NPATCH, DC, 128).transpose(2, 1, 0)).reshape(128, DC * NPATCH)
        qhb = np.ascontiguousarray(
            np.broadcast_to(qhat[None, :], (128, NPATCH)))
        eqm = (QP[b].astype(np.float64) - q8).mean(0)          # mean fp8 residual
        qm = QP[b].astype(np.float64).mean(0)                  # mean query
        sigc = np.square(QP[b].astype(np.float64) - q8).sum(-1).mean() / DPATCH

        for half in range(2):
            sl = slice(half * KEYS, (half + 1) * KEYS)
            k8f = KP[b, sl].astype(F8)
            k8 = k8f.astype(np.float32)
            kk = np.square(k8, dtype=np.float64).sum(-1)
            khat = kk.astype(np.float32)
            k8n = (-k8).astype(F8)
            # stationary: kta[p, kt, c, j] = -k8[kt*128+j, c*128+p]
            kta = np.ascontiguousarray(
                k8n.reshape(KT, 128, DC, 128).transpose(3, 0, 2, 1))

            # analytic fp8 corrections (first+second order)
            ek = KP[b, sl].astype(np.float64) - k8
            g = 1.0 / (qq.mean() + kk + 2 * SMOOTH)
            corr = g * (k8.astype(np.float64) @ eqm) + g * (ek @ qm)
            corr = corr + g ** 2 * (sigc * kk + np.square(ek).sum(-1))

            # sampled per-key reciprocal correction + runtime recip constants
            rows = rng.choice(NPATCH, NSAMP, replace=False)
            qks = q8[rows] @ k8.T
            Ds = (qhat[rows, None] + khat[None, :] - qks).astype(np.float32)
            if consts is None:
                c0, c1 = _optimize_recip_consts(Ds.ravel())
                consts = (c0, c1)
            c0, c1 = consts
            rs = _recip_1nr(Ds, c0, c1).astype(np.float64)
            qks64 = qks.astype(np.float64)
            corr = corr + ((qks64 + SMOOTH) / Ds.astype(np.float64)
                           - qks64 * rs).mean(0)

            cons = np.zeros((128, 4), np.float32)
            cons[:, 0] = c1        # Newton constant  (C0 slot, s0)
            cons[:, 1] = c0        # seed scale       (C1 slot, s1)
            cons[:, 2] = -1.0 / NPATCH   # accumulated sum is -sum(qk*r)
            in_maps.append({
                'qta': qta,
                'kta': kta,
                'qhb': qhb,
                'khat': np.ascontiguousarray(
                    khat.reshape(KT, 128).T),
                'vp': np.ascontiguousarray(VP[b, sl]).astype(ml_dtypes.bfloat16),
                'cons': cons,
                'corr': np.ascontiguousarray(
                    corr.astype(np.float32).reshape(KT, 128).T),
            })
    return in_maps


def _host_finish(outs):
    full = np.empty((B, NPATCH, DPATCH), np.float32)
    for b in range(B):
        full[b, :KEYS] = outs[2 * b]
        full[b, KEYS:] = outs[2 * b + 1]
    return _unpatchify_mat(full)


# --------------------------------------------------------------- bass kernel

def build_nc():
    import concourse.bass as bass  # noqa: F401
    import concourse.mybir as mybir
    import concourse.tile as tile
    from concourse import bacc

    fused_op = _register_fused_op()

    f32 = mybir.dt.float32
    bf16 = mybir.dt.bfloat16
    fp8 = mybir.dt.float8e4
    Alu = mybir.AluOpType
    Act = mybir.ActivationFunctionType
    DR = mybir.MatmulPerfMode.DoubleRow

    nc = bacc.Bacc(
        "TRN2",
        target_bir_lowering=False,
        debug=False,
        enable_asserts=False,
        num_devices=N_CORES,
    )

    qta = nc.dram_tensor("qta", [128, DC * NPATCH], fp8, kind="ExternalInput").ap()
    kta = nc.dram_tensor("kta", [128, KT, DC, 128], fp8, kind="ExternalInput").ap()
    qhb = nc.dram_tensor("qhb", [128, NPATCH], f32, kind="ExternalInput").ap()
    khat = nc.dram_tensor("khat", [128, KT], f32, kind="ExternalInput").ap()
    vp = nc.dram_tensor("vp", [KEYS, DPATCH], bf16, kind="ExternalInput").ap()
    cons = nc.dram_tensor("cons", [128, 4], f32, kind="ExternalInput").ap()
    corr = nc.dram_tensor("corr", [128, KT], f32, kind="ExternalInput").ap()
    out = nc.dram_tensor("out", [KEYS, DPATCH], f32, kind="ExternalOutput").ap()

    with tile.TileContext(nc) as tc:
        with (
            tc.tile_pool(name="ktp", bufs=1) as ktp,
            tc.tile_pool(name="qp", bufs=1) as qp,
            tc.tile_pool(name="qhp", bufs=1) as qhp,
            tc.tile_pool(name="ap_", bufs=4) as ap_,
            tc.tile_pool(name="psp", bufs=2, space="PSUM") as psp,
            tc.tile_pool(name="sop", bufs=4) as sop,
            tc.tile_pool(name="accp", bufs=1) as accp,
            tc.tile_pool(name="wp", bufs=2) as wp,
            tc.tile_pool(name="vvp", bufs=1) as vvp,
            tc.tile_pool(name="outp", bufs=3) as outp,
            tc.tile_pool(name="cnp", bufs=1) as cnp,
        ):
            # --- DMAs -------------------------------------------------------
            # GPSIMD is compute-free AND dma-free (tensor ops + DMA issue on
            # the same Q7 engine crashes it); sync+scalar carry everything.
            # Startup order is arranged to match MM consumption: kt0 keys,
            # then qt0-3 across all 4 contraction pairs split over both
            # queues, qhat rows (needed by the first fused op), qt4-7, rest.
            qta_r = qta.rearrange("p (c i) -> p c i", c=DC)
            # one tile per (chunk-pair, query-half): each has exactly one DMA
            # writer, so matmuls wait on precisely the data they read
            qta_tiles = [
                [qp.tile([128, 2, 2048], fp8, name=f"qta_{cp}_{h}", tag=f"qta{cp}{h}")
                 for h in range(2)]
                for cp in range(DCP)
            ]
            qhb_t = qhp.tile([128, NPATCH], f32, name="qhb_t", tag="qhb")
            cons_t = cnp.tile([128, 4], f32, name="cons_t", tag="cons")
            corr_t = cnp.tile([128, KT], f32, name="corr_t", tag="corr")
            khat_t = cnp.tile([128, KT], f32, name="khat_t", tag="khat")
            kt_tiles = [
                ktp.tile([128, DC, 128], fp8, name=f"kta_{kt}", tag=f"kta{kt}")
                for kt in range(KT)
            ]

            def dma_qta(eng, cp, h, qlo=0, qhi=4):
                qs = slice(h * 2048 + qlo * 512, h * 2048 + qhi * 512)
                eng.dma_start(qta_tiles[cp][h][:, :, qlo * 512:qhi * 512],
                              qta_r[:, 2 * cp:2 * cp + 2, qs])

            # coalesced startup: each dma_start costs ~650ns of queue issue
            # time, so the front of each queue is a few BIG transfers in
            # consumption order; tiny tensors (needed ~15µs in) come later
            nc.sync.dma_start(kt_tiles[0][:, :, :], kta[:, 0, :, :])
            dma_qta(nc.sync, 0, 0, 0, 1)       # first matmul's slice, small
            dma_qta(nc.sync, 0, 0, 1, 4)       # rest of pair 0, qt0-3
            dma_qta(nc.sync, 1, 0)             # pair 1, qt0-3
            nc.sync.dma_start(cons_t[:], cons[:, :])
            nc.sync.dma_start(khat_t[:], khat[:, :])
            nc.sync.dma_start(kt_tiles[1][:, :, :], kta[:, 1, :, :])
            dma_qta(nc.sync, 0, 1)             # pair 0, qt4-7
            dma_qta(nc.sync, 1, 1)             # pair 1, qt4-7
            nc.sync.dma_start(corr_t[:], corr[:, :])
            for kt in range(2, KT):
                nc.sync.dma_start(kt_tiles[kt][:, :, :], kta[:, kt, :, :])

            dma_qta(nc.scalar, 2, 0)           # pair 2, qt0-3
            nc.scalar.dma_start(qhb_t[:, 0:2048], qhb[:, 0:2048])
            dma_qta(nc.scalar, 3, 0)           # pair 3, qt0-3
            dma_qta(nc.scalar, 2, 1)           # pair 2, qt4-7
            dma_qta(nc.scalar, 3, 1)           # pair 3, qt4-7
            nc.scalar.dma_start(qhb_t[:, 2048:], qhb[:, 2048:])

            # values: resident bf16, loaded off the startup critical path
            v_tiles = [
                vvp.tile([128, DPATCH], bf16, name=f"v_{kt}", tag=f"v{kt}")
                for kt in range(KT)
            ]

            acc_tiles = [
                accp.tile([128, 4 if kt == KT - 1 else 2], f32,
                          name=f"acc{kt}", tag=f"acc{kt}")
                for kt in range(KT)
            ]

            def finish_kt(kt):
                red_t = wp.tile([128, 1], f32, name=f"red_{kt}", tag="red")
                nc.vector.tensor_reduce(
                    red_t[:], acc_tiles[kt][:],
                    op=Alu.add, axis=mybir.AxisListType.X)
                w_t = wp.tile([128, 1], f32, name=f"w_{kt}", tag="w")
                nc.vector.scalar_tensor_tensor(
                    w_t[:], red_t[:], cons_t[:, 2:3], corr_t[:, kt:kt + 1],
                    op0=Alu.mult, op1=Alu.add)
                o_t = outp.tile([128, DPATCH], f32, name=f"o_{kt}", tag="o")
                nc.scalar.activation(o_t[:], v_tiles[kt][:], Act.Copy, scale=w_t[:])
                ks = kt * 128
                if kt == KT - 1:
                    # last write-back rides both queues to halve the tail
                    nc.sync.dma_start(out[ks:ks + 64, :], o_t[0:64, :])
                    nc.scalar.dma_start(out[ks + 64:ks + 128, :], o_t[64:128, :])
                else:
                    nc.sync.dma_start(out[ks:ks + 128, :], o_t[:])

            for kt in range(KT):
                # value tiles trickle in ~2 kt ahead of their finish_kt use;
                # v[j]'s DMA must be EMITTED before finish_kt(j) (reads
                # emitted before writes see garbage)
                vjs = ((0, 1, 2) if kt == 0
                       else (kt + 2,) if kt + 2 < KT else ())
                for j in vjs:
                    nc.scalar.dma_start(
                        v_tiles[j][:], vp[j * 128:(j + 1) * 128, :])
                # denominator offset A = qhat + khat[kt] on ACT (Identity with
                # per-partition bias); in halves so the first is ready early
                a_halves = []
                for hh in range(2):
                    at = ap_.tile([128, 2048], f32, name=f"a_{kt}_{hh}", tag="a")
                    nc.scalar.activation(
                        at[:], qhb_t[:, hh * 2048:(hh + 1) * 2048],
                        Act.Identity, bias=khat_t[:, kt:kt + 1], scale=1.0)
                    a_halves.append(at)
                for g in range(2):
                    # one 4-bank PSUM tile per 4-qt group; each matmul's
                    # 512-column output stays within one bank
                    ps = psp.tile([128, 2048], f32, name=f"ps_{kt}_{g}", tag="ps")
                    for c in range(DCP):
                        for qi in range(4):
                            qs = slice(qi * 512, (qi + 1) * 512)
                            nc.tensor.matmul(
                                ps[:, qi * 512:(qi + 1) * 512],
                                kt_tiles[kt][:, 2 * c:2 * c + 2, :],
                                qta_tiles[c][g][:, :, qs],
                                start=(c == 0),
                                stop=(c == DCP - 1),
                                perf_mode=DR,
                            )
                    # one fused recip-MAC over the whole 2048-wide group (the
                    # last kt splits it in two so the tail chain starts sooner)
                    so = sop.tile([128, 2048], bf16, name=f"so_{kt}_{g}", tag="so")
                    if kt == KT - 1:
                        for hh in range(2):
                            sl = slice(hh * 1024, (hh + 1) * 1024)
                            nc.vector._custom_dve(
                                fused_op,
                                out=so[:, sl], in0=ps[:, sl],
                                in1=a_halves[g][:, sl],
                                s0=cons_t[:, 0:1], s1=cons_t[:, 1:2], imm2=0.0,
                                accum_out=acc_tiles[kt][:, 2 * g + hh:
                                                        2 * g + hh + 1],
                            )
                    else:
                        nc.vector._custom_dve(
                            fused_op,
                            out=so[:], in0=ps[:], in1=a_halves[g][:],
                            s0=cons_t[:, 0:1], s1=cons_t[:, 1:2], imm2=0.0,
                            accum_out=acc_tiles[kt][:, g:g + 1],
                        )
                finish_kt(kt)

    nc.compile()
    return nc


_NC_CACHE = None


def _get_nc():
    global _NC_CACHE
    if _NC_CACHE is None:
        _NC_CACHE = build_nc()
    return _NC_CACHE


# ---------------------------------------------------------------- entrypoint

def kernel(q, k, v, _trace=False):
    q = np.asarray(q, dtype=np.float32)
    k = np.asarray(k, dtype=np.float32)
    v = np.asarray(v, dtype=np.float32)

    in_maps = _host_prepare(q, k, v)
    nc = _get_nc()

    from concourse.bass_utils import run_bass_kernel_spmd
    res = None
    for attempt in range(3):
        try:
            res = run_bass_kernel_spmd(
                nc, in_maps, core_ids=list(range(N_CORES)), trace=_trace)
            break
        except Exception:
            if attempt == 2:
                raise
            import time
            time.sleep(2.0)
    outs = [r['out'] for r in res.results]
    result = _host_finish(outs)
    if _trace:
        kernel.last_results = res
    return result


if __name__ == '__main__':
    rng = np.random.default_rng(0)
    q = rng.standard_normal((B, T, C, H, W), dtype=np.float32)
    k = rng.standard_normal((B, T, C, H, W), dtype=np.float32)
    v = rng.standard_normal((B, T, C, H, W), dtype=np.float32)
    o = kernel(q, k, v)
    print("out", o.shape, o.dtype, float(np.abs(o).mean()))
